# revision 1
# baseline (speedup 1.0000x reference)
"""CrossModalityAttention Trainium2 kernel.

Full inputs -> full output; internally shards batch B=8192 across 8 NeuronCores
(pure data parallel). Per core: 1024 samples x K=8 modalities = 8192 tokens of
D=1024.

Device strategy (per core):
  - Host pre-transposes weights to [in,out] (lhsT layout), folds 1/sqrt(128)
    into Wk/bk, folds bv into the residual bias (attention probs sum to 1),
    passes X as XT (d-major fp32r for V; bf16 for Q/K) and XB = x + bo + wo@bv
    (token-major, residual), plus a [128,128] prior/mask table for the
    16-samples-per-128-token-group score layout.
  - Q/K projections + scores run in bf16 (softmax is tolerant); V / O / output
    projection run in fp32r (full PE rate at N>=256, ~2e-4 rel err).
  - Scores are computed transposed per 128-token group: ST[(s,k),(s',q)] via
    matmul(lhsT=Kh^T, rhs=Qh^T); off-diagonal sample pairs get -30 from the
    prior/mask table so exp() kills them; softmax normalization is deferred:
    [O|Z] = exp(ST).T @ [V|1] token-major (ones column folded into V), then
    O *= 1/Z per partition.
  - O is PE-transposed to O^T for the output projection; residual + LayerNorm
    run token-major. rstd = exp(-0.5*ln(var+eps)) keeps every ACT function in
    one table set (no ACT_TABLE_LOAD thrash).
"""

import math

import numpy as np

import concourse.bacc as bacc
import concourse.bass as bass
import concourse.mybir as mybir
import concourse.tile as tile
from concourse.bass_utils import run_bass_kernel_spmd

N_CORES = 8
B, K, D = 8192, 8, 1024
H, HD = 8, 128
BC = B // N_CORES            # samples per core
T = BC * K                   # tokens per core (8192)
TS = 512                     # tokens per tile
NT = T // TS                 # tiles per core
GROUPS = TS // 128           # 128-token groups per tile
SPG = 128 // K               # samples per group (16)
LN_EPS = 1e-5
NEG = -30.0                  # large-negative mask for cross-sample scores

F32 = mybir.dt.float32
F32R = mybir.dt.float32r
BF16 = mybir.dt.bfloat16

_CACHED = None  # compiled Bacc module, built once per process


def _build():
    nc = bacc.Bacc("TRN2", target_bir_lowering=False, debug=False, num_devices=1)

    xtr_d = nc.dram_tensor("XTR", [D, T], F32R, kind="ExternalInput").ap()
    xtb_d = nc.dram_tensor("XTB", [D, T], BF16, kind="ExternalInput").ap()
    xb_d = nc.dram_tensor("XB", [T, D], F32, kind="ExternalInput").ap()
    wq_d = nc.dram_tensor("WQT", [D, D], BF16, kind="ExternalInput").ap()
    wk_d = nc.dram_tensor("WKT", [D, D], BF16, kind="ExternalInput").ap()
    wv_d = nc.dram_tensor("WVT", [D, D], F32R, kind="ExternalInput").ap()
    wo_d = nc.dram_tensor("WOT", [D, D], F32R, kind="ExternalInput").ap()
    bqk_d = nc.dram_tensor("BQK", [128, 2 * H], F32, kind="ExternalInput").ap()
    pm_d = nc.dram_tensor("PM", [128, 128], F32, kind="ExternalInput").ap()
    eye_d = nc.dram_tensor("EYE", [128, 128], F32, kind="ExternalInput").ap()
    ones_d = nc.dram_tensor("ONES1", [128, 1], F32R, kind="ExternalInput").ap()
    out_d = nc.dram_tensor("OUT", [T, D], F32, kind="ExternalOutput").ap()

    xtr_r = xtr_d.rearrange("(c p) t -> p c t", p=128)   # [128, 8, T]
    xtb_r = xtb_d.rearrange("(c p) t -> p c t", p=128)

    with tile.TileContext(nc) as tc:
        with (
            tc.tile_pool(name="wpool", bufs=1) as wpool,
            tc.tile_pool(name="consts", bufs=1) as consts,
            tc.tile_pool(name="xtrp", bufs=1) as xtrp,
            tc.tile_pool(name="xtbp", bufs=2) as xtbp,
            tc.tile_pool(name="qkp", bufs=2) as qkp,
            tc.tile_pool(name="vp", bufs=1) as vp,
            tc.tile_pool(name="ptp", bufs=2) as ptp,
            tc.tile_pool(name="osbp", bufs=1) as osbp,
            tc.tile_pool(name="otp", bufs=2) as otp,
            tc.tile_pool(name="xbp", bufs=2) as xbp,
            tc.tile_pool(name="smalls", bufs=4) as smalls,
            tc.tile_pool(name="projps", bufs=2, space="PSUM") as projps,
            tc.tile_pool(name="attps", bufs=2, space="PSUM") as attps,
            tc.tile_pool(name="zps", bufs=2, space="PSUM") as zps,
        ):
            # ---- constants / weights (resident) ----
            wq = wpool.tile([128, 8, D], BF16, tag="w_q")
            nc.sync.dma_start(wq[:], wq_d.rearrange("(c p) m -> p c m", p=128))
            wk = wpool.tile([128, 8, D], BF16, tag="w_k")
            nc.sync.dma_start(wk[:], wk_d.rearrange("(c p) m -> p c m", p=128))
            wv = wpool.tile([128, 8, D], F32R, tag="w_v")
            nc.sync.dma_start(wv[:], wv_d.rearrange("(c p) m -> p c m", p=128))
            wo = wpool.tile([128, 8, D], F32R, tag="w_o")
            nc.sync.dma_start(wo[:], wo_d.rearrange("(c p) m -> p c m", p=128))
            bqk = consts.tile([128, 2 * H], F32)
            nc.sync.dma_start(bqk[:], bqk_d)
            pm = consts.tile([128, 128], F32)
            nc.sync.dma_start(pm[:], pm_d)
            eye = consts.tile([128, 128], F32)
            nc.sync.dma_start(eye[:], eye_d)
            ones1 = consts.tile([128, 1], F32R)
            nc.sync.dma_start(ones1[:], ones_d)
            eps = consts.tile([128, 1], F32)
            nc.vector.memset(eps[:], LN_EPS)

            for t in range(NT):
                tok0 = t * TS
                xtb = xtbp.tile([128, 8, TS], BF16)
                nc.sync.dma_start(xtb[:], xtb_r[:, :, tok0 : tok0 + TS])
                xtr = xtrp.tile([128, 8, TS], F32R)
                nc.sync.dma_start(xtr[:], xtr_r[:, :, tok0 : tok0 + TS])

                # ---- Q^T, K^T projections (bf16): [d_head(128) x tok(TS)]
                qt = qkp.tile([128, H, TS], BF16, tag="qt")
                kt = qkp.tile([128, H, TS], BF16, tag="kt")
                for wt, dst, bias_col0 in ((wq, qt, 0), (wk, kt, H)):
                    for h in range(H):
                        ps = projps.tile([128, TS], F32, tag="projps")
                        for c in range(8):
                            nc.tensor.matmul(
                                ps[:],
                                wt[:, c, h * HD : (h + 1) * HD],
                                xtb[:, c, :],
                                start=(c == 0),
                                stop=(c == 7),
                            )
                        nc.scalar.activation(
                            dst[:, h, :],
                            ps[:],
                            mybir.ActivationFunctionType.Identity,
                            bias=bqk[:, bias_col0 + h : bias_col0 + h + 1],
                        )

                # ---- V projection (fp32r), token-major
                v = vp.tile([128, GROUPS, H, HD], F32R, tag="v")
                for sub in range(GROUPS):
                    for half in range(2):
                        psv = projps.tile([128, 512], F32, tag="projps")
                        for c in range(8):
                            nc.tensor.matmul(
                                psv[:],
                                xtr[:, c, sub * 128 : (sub + 1) * 128],
                                wv[:, c, half * 512 : (half + 1) * 512],
                                start=(c == 0),
                                stop=(c == 7),
                            )
                        nc.vector.tensor_copy(
                            v[:, sub, 4 * half : 4 * half + 4, :],
                            psv.rearrange("p (a b) -> p a b", a=4),
                        )

                # ---- attention + output proj + residual + LN per 128-tok group
                for g in range(GROUPS):
                    gsl = slice(g * 128, (g + 1) * 128)
                    st = attps.tile([128, H, 128], F32, tag="attps")
                    for h in range(H):
                        nc.tensor.matmul(st[:, h, :], kt[:, h, gsl], qt[:, h, gsl])
                    # add prior/mask (same [128,128] table per head), in place
                    nc.vector.tensor_tensor(
                        st[:],
                        st[:],
                        pm[:, None, :].to_broadcast((128, H, 128)),
                        mybir.AluOpType.add,
                    )
                    pt = ptp.tile([128, H, 128], F32R)
                    nc.scalar.activation(
                        pt[:], st[:], mybir.ActivationFunctionType.Exp
                    )
                    oz = attps.tile([128, H, 128], F32, tag="attps")
                    zp = zps.tile([128, H], F32)
                    for h in range(H):
                        nc.tensor.matmul(oz[:, h, :], pt[:, h, :], v[:, g, h, :])
                        # fp32r rejects N=1 dst patterns; bitcast to plain f32
                        nc.tensor.matmul(
                            zp[:, h : h + 1],
                            pt[:, h, :].bitcast(F32),
                            ones1[:].bitcast(F32),
                        )
                    rz = smalls.tile([128, H], F32, tag="rz")
                    nc.vector.reciprocal(rz[:], zp[:])
                    osb = osbp.tile([128, H, HD], F32)
                    nc.vector.tensor_tensor(
                        osb[:],
                        oz[:],
                        rz[:, :, None].to_broadcast((128, H, HD)),
                        mybir.AluOpType.mult,
                    )
                    tp = attps.tile([128, H, 128], F32, tag="attps")
                    for h in range(H):
                        nc.tensor.transpose(tp[:, h, :], osb[:, h, :], eye[:])
                    ot = otp.tile([128, H, 128], F32R)
                    nc.scalar.activation(
                        ot[:], tp[:], mybir.ActivationFunctionType.Copy
                    )

                    xb = xbp.tile([128, D], F32)
                    nc.sync.dma_start(
                        xb[:], xb_d[tok0 + g * 128 : tok0 + (g + 1) * 128, :]
                    )
                    for half in range(2):
                        yp = projps.tile([128, 512], F32, tag="projps")
                        for c in range(8):
                            nc.tensor.matmul(
                                yp[:],
                                ot[:, c, :],
                                wo[:, c, half * 512 : (half + 1) * 512],
                                start=(c == 0),
                                stop=(c == 7),
                            )
                        nc.vector.tensor_tensor(
                            xb[:, half * 512 : (half + 1) * 512],
                            xb[:, half * 512 : (half + 1) * 512],
                            yp[:],
                            mybir.AluOpType.add,
                        )
                    stats = smalls.tile([128, 2, 6], F32, tag="stats")
                    for sg in range(2):
                        nc.vector.bn_stats(
                            stats[:, sg, :], xb[:, sg * 512 : (sg + 1) * 512]
                        )
                    mv = smalls.tile([128, 2], F32, tag="mv")
                    nc.vector.bn_aggr(mv[:], stats[:])
                    # rstd = exp(-0.5*ln(var+eps)); ln+exp live in one ACT
                    # table set (sqrt does not), avoiding table reloads
                    sd = smalls.tile([128, 1], F32, tag="sd")
                    nc.scalar.activation(
                        sd[:],
                        mv[:, 1:2],
                        mybir.ActivationFunctionType.Ln,
                        bias=eps[:],
                    )
                    nc.scalar.activation(
                        sd[:], sd[:], mybir.ActivationFunctionType.Exp, scale=-0.5
                    )
                    nc.vector.tensor_scalar(
                        out=xb[:],
                        in0=xb[:],
                        scalar1=mv[:, 0:1],
                        scalar2=sd[:],
                        op0=mybir.AluOpType.subtract,
                        op1=mybir.AluOpType.mult,
                    )
                    nc.sync.dma_start(
                        out_d[tok0 + g * 128 : tok0 + (g + 1) * 128, :], xb[:]
                    )

    nc.compile()
    return nc


def _get_nc():
    global _CACHED
    if _CACHED is None:
        _CACHED = _build()
    return _CACHED


def _reference_numpy(modality_encodings, selection_mask, wq, bq, wk, bk, wv, bv,
                     wo, bo, rel_prior, ln_gamma, ln_beta):
    """Slow fallback, exact port of the reference (used only if inputs fall
    outside the fast path's assumptions: non-trivial mask)."""
    x = modality_encodings.astype(np.float32)
    Bn, Kn, Dn = x.shape
    Hd = Dn // H
    q = (x @ wq.T + bq).reshape(Bn, Kn, H, Hd).transpose(0, 2, 1, 3)
    k = (x @ wk.T + bk).reshape(Bn, Kn, H, Hd).transpose(0, 2, 1, 3)
    v = (x @ wv.T + bv).reshape(Bn, Kn, H, Hd).transpose(0, 2, 1, 3)
    scores = np.einsum("bhqd,bhkd->bhqk", q, k) / math.sqrt(Hd)
    scores = scores + rel_prior[None, None]
    mask2d = (selection_mask[:, :, None] * selection_mask[:, None, :]) > 0
    scores = np.where(mask2d[:, None], scores, -np.inf)
    scores = scores - scores.max(axis=-1, keepdims=True)
    e = np.exp(scores)
    attn = e / e.sum(axis=-1, keepdims=True)
    out = np.einsum("bhqk,bhkd->bhqd", attn, v)
    out = out.transpose(0, 2, 1, 3).reshape(Bn, Kn, Dn)
    out = out @ wo.T + bo
    res = x + out
    mu = res.mean(-1, keepdims=True)
    var = ((res - mu) ** 2).mean(-1, keepdims=True)
    return (res - mu) / np.sqrt(var + LN_EPS) * ln_gamma + ln_beta


def _prep_in_maps(modality_encodings, wq, bq, wk, bk, wv, bv, wo, bo, rel_prior):
    import ml_dtypes

    s = 1.0 / math.sqrt(HD)
    wqt = np.ascontiguousarray(wq.T).astype(ml_dtypes.bfloat16)
    wkt = np.ascontiguousarray((wk * s).T).astype(ml_dtypes.bfloat16)
    wvt = np.ascontiguousarray(wv.T)
    wot = np.ascontiguousarray(wo.T)
    bks = bk * s
    b_eff = (bo + wo @ bv).astype(np.float32)

    bqk = np.concatenate(
        [bq.reshape(H, HD).T, bks.reshape(H, HD).T], axis=1
    ).astype(np.float32)  # [128, 16]

    pmat = np.full((128, 128), NEG, dtype=np.float32)
    for sm in range(SPG):
        pmat[sm * K : (sm + 1) * K, sm * K : (sm + 1) * K] = rel_prior.T
    eye = np.eye(128, dtype=np.float32)
    ones1 = np.ones((128, 1), dtype=np.float32)

    x_flat = modality_encodings.reshape(B * K, D)
    in_maps = []
    for c in range(N_CORES):
        x_core = x_flat[c * T : (c + 1) * T]
        xt = np.ascontiguousarray(x_core.T)
        in_maps.append({
            "XTR": xt,
            "XTB": xt.astype(ml_dtypes.bfloat16),
            "XB": x_core + b_eff,
            "WQT": wqt, "WKT": wkt, "WVT": wvt, "WOT": wot,
            "BQK": bqk, "PM": pmat, "EYE": eye, "ONES1": ones1,
        })
    return in_maps


def run_device(inputs, trace=False):
    """Build in_maps from full inputs, run on 8 cores, return (full_out, results)."""
    in_maps = _prep_in_maps(
        inputs["modality_encodings"], inputs["wq"], inputs["bq"], inputs["wk"],
        inputs["bk"], inputs["wv"], inputs["bv"], inputs["wo"], inputs["bo"],
        inputs["rel_prior"],
    )
    nc = _get_nc()
    res = run_bass_kernel_spmd(nc, in_maps, core_ids=list(range(N_CORES)), trace=trace)
    out = np.concatenate(
        [res.results[c]["OUT"].reshape(BC, K, D) for c in range(N_CORES)], axis=0
    )
    return out, res


def kernel(**inputs) -> np.ndarray:
    inputs = {k: np.asarray(v) for k, v in inputs.items()}
    mask = inputs["selection_mask"]
    gamma = inputs["ln_gamma"]
    beta = inputs["ln_beta"]
    if not np.all(mask > 0):
        # general-mask fallback (never hit for the spec'd inputs: fill=ones)
        return _reference_numpy(**{k: inputs[k].astype(np.float32) for k in (
            "modality_encodings", "selection_mask", "wq", "bq", "wk", "bk",
            "wv", "bv", "wo", "bo", "rel_prior", "ln_gamma", "ln_beta")}
        ).astype(np.float32)

    out, _ = run_device(inputs, trace=False)
    # device kernel skips the (identity for spec'd inputs) LN affine params
    if not (np.all(gamma == 1.0) and np.all(beta == 0.0)):
        out = out * gamma + beta
    return out.astype(np.float32)



# revision 2
# speedup vs baseline: 1.3100x; 1.3100x over previous
"""CrossModalityAttention Trainium2 kernel (v2: fp8 DoubleRow projections).

Full inputs -> full output; internally shards batch B=8192 across 8 NeuronCores
(pure data parallel). Per core: 1024 samples x K=8 modalities = 8192 tokens of
D=1024.

Device strategy (per core):
  - All four DxD projections run in fp8-e4m3 with perf_mode=DoubleRow (2
    fp8 MACs/cell/cycle, contraction chunk pairs packed in the AP's dim1).
    Power-of-2 quantization scales: x*32, wq/wv/wo*4096, (wk/sqrt(128))*32768,
    attention-output*32. Descales fold into the ACT bias stage (Q/K/V) or the
    2^17-prescaled residual XB (output proj), whose scale LayerNorm absorbs
    exactly via eps' = eps*2^34 (LN is scale-invariant).
  - Scores per 128-token group (16 samples x K=8) stay bf16:
    ST[(s,k),(s',q)] = Kh^T Qh via matmul(lhsT=kt, rhs=qt); prior/mask table
    adds rel_prior on the block diagonal and -30 off it; exp() -> bf16 probs.
  - Transpose-free tail: OT[d,q] = matmul(lhsT=V[k,d] bf16, rhs=P[k,q] bf16)
    gives the output projection's lhsT directly (no PE transposes). The
    softmax denominator reaches all 128 partitions via one rank-1 PE matmul
    ZB = (1/32 ones)^T @ P, then DVE reciprocal + multiply quantize OT to
    fp8 (the 1/32 folds the fp8 scale for free).
  - LayerNorm: rstd = exp(-0.5*ln(var+eps')) keeps every ACT function in one
    table set.
"""

import math

import numpy as np

import concourse.bacc as bacc
import concourse.bass as bass
import concourse.mybir as mybir
import concourse.tile as tile
from concourse.bass_utils import run_bass_kernel_spmd

N_CORES = 8
B, K, D = 8192, 8, 1024
H, HD = 8, 128
BC = B // N_CORES            # samples per core
T = BC * K                   # tokens per core (8192)
TS = 1024                    # tokens per tile
NT = T // TS                 # tiles per core
GROUPS = TS // 128           # 128-token groups per tile
SPG = 128 // K               # samples per group (16)
LN_EPS = 1e-5
NEG = -30.0                  # large-negative mask for cross-sample scores

XS = 32.0                    # fp8 scale for x and attention output
WS = 4096.0                  # fp8 scale for wq/wv/wo
KS = 32768.0                 # fp8 scale for wk/sqrt(HD)
RS = float(2 ** 17)          # residual prescale (= XS*WS); LN absorbs it

F32 = mybir.dt.float32
BF16 = mybir.dt.bfloat16
FP8 = mybir.dt.float8e4
DR = mybir.MatmulPerfMode.DoubleRow

_CACHED = None  # compiled Bacc module, built once per process


def _build():
    nc = bacc.Bacc("TRN2", target_bir_lowering=False, debug=False, num_devices=1)

    xt8_d = nc.dram_tensor("XT8", [D, T], FP8, kind="ExternalInput").ap()
    xb_d = nc.dram_tensor("XB", [T, D], F32, kind="ExternalInput").ap()
    wq_d = nc.dram_tensor("WQ8", [D, D], FP8, kind="ExternalInput").ap()
    wk_d = nc.dram_tensor("WK8", [D, D], FP8, kind="ExternalInput").ap()
    wv_d = nc.dram_tensor("WV8", [D, D], FP8, kind="ExternalInput").ap()
    wo_d = nc.dram_tensor("WO8", [D, D], FP8, kind="ExternalInput").ap()
    bqk_d = nc.dram_tensor("BQK", [128, 2 * H], F32, kind="ExternalInput").ap()
    pm_d = nc.dram_tensor("PM", [128, 128], F32, kind="ExternalInput").ap()
    om_d = nc.dram_tensor("OM", [128, 128], BF16, kind="ExternalInput").ap()
    out_d = nc.dram_tensor("OUT", [T, D], F32, kind="ExternalOutput").ap()

    xt8_r = xt8_d.rearrange("(c p) t -> p c t", p=128)   # [128, 8, T]

    with tile.TileContext(nc) as tc:
        with (
            tc.tile_pool(name="wpool", bufs=1) as wpool,
            tc.tile_pool(name="consts", bufs=1) as consts,
            tc.tile_pool(name="xt8p", bufs=2) as xt8p,
            tc.tile_pool(name="qkp", bufs=2) as qkp,
            tc.tile_pool(name="vp", bufs=1) as vp,
            tc.tile_pool(name="ptp", bufs=2) as ptp,
            tc.tile_pool(name="rzbp", bufs=2) as rzbp,
            tc.tile_pool(name="ot8p", bufs=2) as ot8p,
            tc.tile_pool(name="xbp", bufs=2) as xbp,
            tc.tile_pool(name="smalls", bufs=4) as smalls,
            tc.tile_pool(name="projps", bufs=4, space="PSUM") as projps,
            tc.tile_pool(name="attps", bufs=1, space="PSUM") as attps,
            tc.tile_pool(name="otps", bufs=1, space="PSUM") as otps,
        ):
            # ---- constants / weights (resident) ----
            wq = wpool.tile([128, 8, D], FP8, tag="w_q")
            nc.sync.dma_start(wq[:], wq_d.rearrange("(c p) m -> p c m", p=128))
            wk = wpool.tile([128, 8, D], FP8, tag="w_k")
            nc.sync.dma_start(wk[:], wk_d.rearrange("(c p) m -> p c m", p=128))
            wv = wpool.tile([128, 8, D], FP8, tag="w_v")
            nc.sync.dma_start(wv[:], wv_d.rearrange("(c p) m -> p c m", p=128))
            wo = wpool.tile([128, 8, D], FP8, tag="w_o")
            nc.sync.dma_start(wo[:], wo_d.rearrange("(c p) m -> p c m", p=128))
            bqk = consts.tile([128, 2 * H], F32)
            nc.sync.dma_start(bqk[:], bqk_d)
            pm = consts.tile([128, 128], F32)
            nc.sync.dma_start(pm[:], pm_d)
            om = consts.tile([128, 128], BF16)
            nc.sync.dma_start(om[:], om_d)
            eps = consts.tile([128, 1], F32)
            nc.vector.memset(eps[:], LN_EPS * RS * RS)

            for t in range(NT):
                tok0 = t * TS
                xt8 = xt8p.tile([128, 8, TS], FP8)
                nc.sync.dma_start(xt8[:], xt8_r[:, :, tok0 : tok0 + TS])

                # ---- Q^T, K^T projections (fp8 DoubleRow): [d_head x tok]
                qt = qkp.tile([128, H, TS], BF16, tag="qt")
                kt = qkp.tile([128, H, TS], BF16, tag="kt")
                for wt, dst, bias_col0, dsc in (
                    (wq, qt, 0, 1.0 / RS),
                    (wk, kt, H, 1.0 / (KS * XS)),
                ):
                    for h in range(H):
                        psa = projps.tile([128, 512], F32, tag="projps")
                        psb = projps.tile([128, 512], F32, tag="projps")
                        for c in range(4):
                            lw = wt[:, 2 * c : 2 * c + 2, h * HD : (h + 1) * HD]
                            nc.tensor.matmul(
                                psa[:], lw, xt8[:, 2 * c : 2 * c + 2, 0:512],
                                start=(c == 0), stop=(c == 3), perf_mode=DR,
                            )
                            nc.tensor.matmul(
                                psb[:], lw, xt8[:, 2 * c : 2 * c + 2, 512:1024],
                                start=(c == 0), stop=(c == 3), perf_mode=DR,
                            )
                        bias = bqk[:, bias_col0 + h : bias_col0 + h + 1]
                        nc.scalar.activation(
                            dst[:, h, 0:512], psa[:],
                            mybir.ActivationFunctionType.Identity,
                            bias=bias, scale=dsc,
                        )
                        nc.scalar.activation(
                            dst[:, h, 512:1024], psb[:],
                            mybir.ActivationFunctionType.Identity,
                            bias=bias, scale=dsc,
                        )

                # ---- V projection (fp8 DoubleRow), token-major bf16
                v = vp.tile([128, GROUPS, H, HD], BF16, tag="v")
                for sub in range(GROUPS):
                    psv0 = projps.tile([128, 512], F32, tag="projps")
                    psv1 = projps.tile([128, 512], F32, tag="projps")
                    for c in range(4):
                        lx = xt8[:, 2 * c : 2 * c + 2, sub * 128 : (sub + 1) * 128]
                        nc.tensor.matmul(
                            psv0[:], lx, wv[:, 2 * c : 2 * c + 2, 0:512],
                            start=(c == 0), stop=(c == 3), perf_mode=DR,
                        )
                        nc.tensor.matmul(
                            psv1[:], lx, wv[:, 2 * c : 2 * c + 2, 512:1024],
                            start=(c == 0), stop=(c == 3), perf_mode=DR,
                        )
                    nc.scalar.activation(
                        v[:, sub, 0:4, :], psv0.rearrange("p (a b) -> p a b", a=4),
                        mybir.ActivationFunctionType.Copy, scale=1.0 / RS,
                    )
                    nc.scalar.activation(
                        v[:, sub, 4:8, :], psv1.rearrange("p (a b) -> p a b", a=4),
                        mybir.ActivationFunctionType.Copy, scale=1.0 / RS,
                    )

                # ---- attention + output proj + residual + LN per 128-tok group
                for g in range(GROUPS):
                    gsl = slice(g * 128, (g + 1) * 128)
                    st = attps.tile([128, H, 128], F32, tag="attps")
                    for h in range(H):
                        nc.tensor.matmul(st[:, h, :], kt[:, h, gsl], qt[:, h, gsl])
                    # add prior/mask (same [128,128] table per head), in place
                    nc.vector.tensor_tensor(
                        st[:], st[:],
                        pm[:, None, :].to_broadcast((128, H, 128)),
                        mybir.AluOpType.add,
                    )
                    pt = ptp.tile([128, H, 128], BF16)
                    nc.scalar.activation(
                        pt[:], st[:], mybir.ActivationFunctionType.Exp
                    )
                    # denominator, broadcast to all partitions: ZB = (1/32)^T P
                    zb0 = projps.tile([128, 512], F32, tag="projps")
                    zb1 = projps.tile([128, 512], F32, tag="projps")
                    nc.tensor.matmul(zb0[:], om[:], pt[:, 0:4, :])
                    nc.tensor.matmul(zb1[:], om[:], pt[:, 4:8, :])
                    rzb = rzbp.tile([128, H, 128], F32)
                    nc.vector.reciprocal(
                        rzb[:, 0:4, :], zb0.rearrange("p (a b) -> p a b", a=4)
                    )
                    nc.vector.reciprocal(
                        rzb[:, 4:8, :], zb1.rearrange("p (a b) -> p a b", a=4)
                    )
                    # OT[d, q] = V^T P per head (lhsT=v), then *32/Z -> fp8
                    ot = otps.tile([128, H, 128], F32, tag="otps")
                    for h in range(H):
                        nc.tensor.matmul(ot[:, h, :], v[:, g, h, :], pt[:, h, :])
                    ot8 = ot8p.tile([128, H, 128], FP8)
                    nc.vector.tensor_tensor(
                        ot8[:], ot[:], rzb[:], mybir.AluOpType.mult
                    )

                    xb = xbp.tile([128, D], F32)
                    nc.sync.dma_start(
                        xb[:], xb_d[tok0 + g * 128 : tok0 + (g + 1) * 128, :]
                    )
                    yp0 = projps.tile([128, 512], F32, tag="projps")
                    yp1 = projps.tile([128, 512], F32, tag="projps")
                    for c in range(4):
                        lo = ot8[:, 2 * c : 2 * c + 2, :]
                        nc.tensor.matmul(
                            yp0[:], lo, wo[:, 2 * c : 2 * c + 2, 0:512],
                            start=(c == 0), stop=(c == 3), perf_mode=DR,
                        )
                        nc.tensor.matmul(
                            yp1[:], lo, wo[:, 2 * c : 2 * c + 2, 512:1024],
                            start=(c == 0), stop=(c == 3), perf_mode=DR,
                        )
                    nc.vector.tensor_tensor(
                        xb[:, 0:512], xb[:, 0:512], yp0[:], mybir.AluOpType.add
                    )
                    nc.vector.tensor_tensor(
                        xb[:, 512:1024], xb[:, 512:1024], yp1[:],
                        mybir.AluOpType.add,
                    )
                    stats = smalls.tile([128, 2, 6], F32, tag="stats")
                    for sg in range(2):
                        nc.vector.bn_stats(
                            stats[:, sg, :], xb[:, sg * 512 : (sg + 1) * 512]
                        )
                    mv = smalls.tile([128, 2], F32, tag="mv")
                    nc.vector.bn_aggr(mv[:], stats[:])
                    # rstd = exp(-0.5*ln(var+eps')); ln+exp live in one ACT
                    # table set (sqrt does not), avoiding table reloads
                    sd = smalls.tile([128, 1], F32, tag="sd")
                    nc.scalar.activation(
                        sd[:], mv[:, 1:2],
                        mybir.ActivationFunctionType.Ln, bias=eps[:],
                    )
                    nc.scalar.activation(
                        sd[:], sd[:], mybir.ActivationFunctionType.Exp, scale=-0.5
                    )
                    nc.vector.tensor_scalar(
                        out=xb[:],
                        in0=xb[:],
                        scalar1=mv[:, 0:1],
                        scalar2=sd[:],
                        op0=mybir.AluOpType.subtract,
                        op1=mybir.AluOpType.mult,
                    )
                    nc.sync.dma_start(
                        out_d[tok0 + g * 128 : tok0 + (g + 1) * 128, :], xb[:]
                    )

    nc.compile()
    return nc


def _get_nc():
    global _CACHED
    if _CACHED is None:
        _CACHED = _build()
    return _CACHED


def _reference_numpy(modality_encodings, selection_mask, wq, bq, wk, bk, wv, bv,
                     wo, bo, rel_prior, ln_gamma, ln_beta):
    """Slow fallback, exact port of the reference (used only if inputs fall
    outside the fast path's assumptions: non-trivial mask)."""
    x = modality_encodings.astype(np.float32)
    Bn, Kn, Dn = x.shape
    Hd = Dn // H
    q = (x @ wq.T + bq).reshape(Bn, Kn, H, Hd).transpose(0, 2, 1, 3)
    k = (x @ wk.T + bk).reshape(Bn, Kn, H, Hd).transpose(0, 2, 1, 3)
    v = (x @ wv.T + bv).reshape(Bn, Kn, H, Hd).transpose(0, 2, 1, 3)
    scores = np.einsum("bhqd,bhkd->bhqk", q, k) / math.sqrt(Hd)
    scores = scores + rel_prior[None, None]
    mask2d = (selection_mask[:, :, None] * selection_mask[:, None, :]) > 0
    scores = np.where(mask2d[:, None], scores, -np.inf)
    scores = scores - scores.max(axis=-1, keepdims=True)
    e = np.exp(scores)
    attn = e / e.sum(axis=-1, keepdims=True)
    out = np.einsum("bhqk,bhkd->bhqd", attn, v)
    out = out.transpose(0, 2, 1, 3).reshape(Bn, Kn, Dn)
    out = out @ wo.T + bo
    res = x + out
    mu = res.mean(-1, keepdims=True)
    var = ((res - mu) ** 2).mean(-1, keepdims=True)
    return (res - mu) / np.sqrt(var + LN_EPS) * ln_gamma + ln_beta


def _q8(a, scale):
    import ml_dtypes

    return np.clip(a * scale, -240.0, 240.0).astype(ml_dtypes.float8_e4m3)


def _prep_in_maps(modality_encodings, wq, bq, wk, bk, wv, bv, wo, bo, rel_prior):
    import ml_dtypes

    s = 1.0 / math.sqrt(HD)
    wq8 = _q8(np.ascontiguousarray(wq.T), WS)
    wk8 = _q8(np.ascontiguousarray((wk * s).T), KS)
    wv8 = _q8(np.ascontiguousarray(wv.T), WS)
    wo8 = _q8(np.ascontiguousarray(wo.T), WS)
    bks = bk * s
    b_eff = (bo + wo @ bv).astype(np.float32)

    bqk = np.concatenate(
        [bq.reshape(H, HD).T, bks.reshape(H, HD).T], axis=1
    ).astype(np.float32)  # [128, 16]

    pmat = np.full((128, 128), NEG, dtype=np.float32)
    for sm in range(SPG):
        pmat[sm * K : (sm + 1) * K, sm * K : (sm + 1) * K] = rel_prior.T
    omat = np.full((128, 128), 1.0 / XS, dtype=ml_dtypes.bfloat16)

    x_flat = modality_encodings.reshape(B * K, D)
    in_maps = []
    for c in range(N_CORES):
        x_core = x_flat[c * T : (c + 1) * T]
        in_maps.append({
            "XT8": _q8(np.ascontiguousarray(x_core.T), XS),
            "XB": (x_core + b_eff) * RS,
            "WQ8": wq8, "WK8": wk8, "WV8": wv8, "WO8": wo8,
            "BQK": bqk, "PM": pmat, "OM": omat,
        })
    return in_maps


def run_device(inputs, trace=False):
    """Build in_maps from full inputs, run on 8 cores, return (full_out, results)."""
    in_maps = _prep_in_maps(
        inputs["modality_encodings"], inputs["wq"], inputs["bq"], inputs["wk"],
        inputs["bk"], inputs["wv"], inputs["bv"], inputs["wo"], inputs["bo"],
        inputs["rel_prior"],
    )
    nc = _get_nc()
    res = run_bass_kernel_spmd(nc, in_maps, core_ids=list(range(N_CORES)), trace=trace)
    out = np.concatenate(
        [res.results[c]["OUT"].reshape(BC, K, D) for c in range(N_CORES)], axis=0
    )
    return out, res


def kernel(**inputs) -> np.ndarray:
    inputs = {k: np.asarray(v) for k, v in inputs.items()}
    mask = inputs["selection_mask"]
    gamma = inputs["ln_gamma"]
    beta = inputs["ln_beta"]
    if not np.all(mask > 0):
        # general-mask fallback (never hit for the spec'd inputs: fill=ones)
        return _reference_numpy(**{k: inputs[k].astype(np.float32) for k in (
            "modality_encodings", "selection_mask", "wq", "bq", "wk", "bk",
            "wv", "bv", "wo", "bo", "rel_prior", "ln_gamma", "ln_beta")}
        ).astype(np.float32)

    out, _ = run_device(inputs, trace=False)
    # device kernel skips the (identity for spec'd inputs) LN affine params
    if not (np.all(gamma == 1.0) and np.all(beta == 0.0)):
        out = out * gamma + beta
    return out.astype(np.float32)


# revision 6
# speedup vs baseline: 1.7385x; 1.3270x over previous
"""CrossModalityAttention Trainium2 kernel (v2: fp8 DoubleRow projections).

Full inputs -> full output; internally shards batch B=8192 across 8 NeuronCores
(pure data parallel). Per core: 1024 samples x K=8 modalities = 8192 tokens of
D=1024.

Device strategy (per core):
  - All four DxD projections run in fp8-e4m3 with perf_mode=DoubleRow (2
    fp8 MACs/cell/cycle, contraction chunk pairs packed in the AP's dim1).
    Power-of-2 quantization scales: x*32, wq/wv/wo*4096, (wk/sqrt(128))*32768,
    attention-output*32. Descales fold into the ACT bias stage (Q/K/V) or the
    2^17-prescaled residual XB (output proj), whose scale LayerNorm absorbs
    exactly via eps' = eps*2^34 (LN is scale-invariant).
  - Scores per 128-token group (16 samples x K=8) stay bf16:
    ST[(s,k),(s',q)] = Kh^T Qh via matmul(lhsT=kt, rhs=qt); prior/mask table
    adds rel_prior on the block diagonal and -30 off it; exp() -> bf16 probs.
  - Transpose-free tail: OT[d,q] = matmul(lhsT=V[k,d] bf16, rhs=P[k,q] bf16)
    gives the output projection's lhsT directly (no PE transposes). The
    softmax denominator reaches all 128 partitions via one rank-1 PE matmul
    ZB = (1/32 ones)^T @ P, then DVE reciprocal + multiply quantize OT to
    fp8 (the 1/32 folds the fp8 scale for free).
  - LayerNorm: rstd = exp(-0.5*ln(var+eps')) keeps every ACT function in one
    table set.
"""

import math

import numpy as np

import concourse.bacc as bacc
import concourse.bass as bass
import concourse.mybir as mybir
import concourse.tile as tile
from concourse.bass_utils import run_bass_kernel_spmd

N_CORES = 8
B, K, D = 8192, 8, 1024
H, HD = 8, 128
BC = B // N_CORES            # samples per core
T = BC * K                   # tokens per core (8192)
TS = 1024                    # tokens per tile
NT = T // TS                 # tiles per core
GROUPS = TS // 128           # 128-token groups per tile
SPG = 128 // K               # samples per group (16)
LN_EPS = 1e-5
NEG = -30.0                  # large-negative mask for cross-sample scores

XS = 32.0                    # fp8 scale for x and attention output
WS = 4096.0                  # fp8 scale for wq/wv/wo
KS = 32768.0                 # fp8 scale for wk/sqrt(HD)
RS = float(2 ** 17)          # residual prescale (= XS*WS); LN absorbs it

F32 = mybir.dt.float32
BF16 = mybir.dt.bfloat16
FP8 = mybir.dt.float8e4
DR = mybir.MatmulPerfMode.DoubleRow

_CACHED = None  # compiled Bacc module, built once per process


def _build():
    nc = bacc.Bacc("TRN2", target_bir_lowering=False, debug=False, num_devices=1)

    xt8_d = nc.dram_tensor("XT8", [D, T], FP8, kind="ExternalInput").ap()
    xb_d = nc.dram_tensor("XB", [T, D], F32, kind="ExternalInput").ap()
    wq_d = nc.dram_tensor("WQ8", [D, D], FP8, kind="ExternalInput").ap()
    wk_d = nc.dram_tensor("WK8", [D, D], FP8, kind="ExternalInput").ap()
    wv_d = nc.dram_tensor("WV8", [D, D], FP8, kind="ExternalInput").ap()
    wo_d = nc.dram_tensor("WO8", [D, D], FP8, kind="ExternalInput").ap()
    bqk_d = nc.dram_tensor("BQK", [128, 2 * H], F32, kind="ExternalInput").ap()
    pm_d = nc.dram_tensor("PM", [128, 128], F32, kind="ExternalInput").ap()
    om_d = nc.dram_tensor("OM", [128, 128], BF16, kind="ExternalInput").ap()
    out_d = nc.dram_tensor("OUT", [T, D], F32, kind="ExternalOutput").ap()

    xt8_r = xt8_d.rearrange("(c p) t -> p c t", p=128)   # [128, 8, T]

    with tile.TileContext(nc) as tc:
        with (
            tc.tile_pool(name="wpool", bufs=1) as wpool,
            tc.tile_pool(name="consts", bufs=1) as consts,
            tc.tile_pool(name="xt8p", bufs=2) as xt8p,
            tc.tile_pool(name="qkp", bufs=2) as qkp,
            tc.tile_pool(name="vp", bufs=1) as vp,
            tc.tile_pool(name="ptp", bufs=2) as ptp,
            tc.tile_pool(name="rzbp", bufs=2) as rzbp,
            tc.tile_pool(name="ot8p", bufs=2) as ot8p,
            tc.tile_pool(name="xbp", bufs=GROUPS + 1) as xbp,
            tc.tile_pool(name="smalls", bufs=4) as smalls,
            tc.tile_pool(name="projps", bufs=4, space="PSUM") as projps,
            tc.tile_pool(name="attps", bufs=1, space="PSUM") as attps,
            tc.tile_pool(name="otps", bufs=1, space="PSUM") as otps,
        ):
            # ---- constants / weights (resident) ----
            wq = wpool.tile([128, 8, D], FP8, tag="w_q")
            nc.sync.dma_start(wq[:], wq_d.rearrange("(c p) m -> p c m", p=128))
            wk = wpool.tile([128, 8, D], FP8, tag="w_k")
            nc.sync.dma_start(wk[:], wk_d.rearrange("(c p) m -> p c m", p=128))
            wv = wpool.tile([128, 8, D], FP8, tag="w_v")
            nc.sync.dma_start(wv[:], wv_d.rearrange("(c p) m -> p c m", p=128))
            wo = wpool.tile([128, 8, D], FP8, tag="w_o")
            nc.sync.dma_start(wo[:], wo_d.rearrange("(c p) m -> p c m", p=128))
            bqk = consts.tile([128, 2 * H], F32)
            nc.sync.dma_start(bqk[:], bqk_d)
            pm = consts.tile([128, 128], F32)
            nc.sync.dma_start(pm[:], pm_d)
            om = consts.tile([128, 128], BF16)
            nc.sync.dma_start(om[:], om_d)
            eps = consts.tile([128, 1], F32)
            nc.vector.memset(eps[:], LN_EPS * RS * RS)

            for t in range(NT):
                tok0 = t * TS
                xt8 = xt8p.tile([128, 8, TS], FP8)
                nc.sync.dma_start(xt8[:], xt8_r[:, :, tok0 : tok0 + TS])

                # ---- Q^T, K^T projections (fp8 DoubleRow): [d_head x tok]
                qt = qkp.tile([128, H, TS], BF16, tag="qt")
                kt = qkp.tile([128, H, TS], BF16, tag="kt")
                for wt, dst, bias_col0, dsc in (
                    (wq, qt, 0, 1.0 / RS),
                    (wk, kt, H, 1.0 / (KS * XS)),
                ):
                    for h in range(H):
                        psa = projps.tile([128, 512], F32, tag="projps")
                        psb = projps.tile([128, 512], F32, tag="projps")
                        for c in range(4):
                            lw = wt[:, 2 * c : 2 * c + 2, h * HD : (h + 1) * HD]
                            nc.tensor.matmul(
                                psa[:], lw, xt8[:, 2 * c : 2 * c + 2, 0:512],
                                start=(c == 0), stop=(c == 3), perf_mode=DR,
                            )
                            nc.tensor.matmul(
                                psb[:], lw, xt8[:, 2 * c : 2 * c + 2, 512:1024],
                                start=(c == 0), stop=(c == 3), perf_mode=DR,
                            )
                        bias = bqk[:, bias_col0 + h : bias_col0 + h + 1]
                        nc.scalar.activation(
                            dst[:, h, 0:512], psa[:],
                            mybir.ActivationFunctionType.Identity,
                            bias=bias, scale=dsc,
                        )
                        nc.scalar.activation(
                            dst[:, h, 512:1024], psb[:],
                            mybir.ActivationFunctionType.Identity,
                            bias=bias, scale=dsc,
                        )

                # ---- V projection (fp8 DoubleRow), token-major bf16
                v = vp.tile([128, GROUPS, H, HD], BF16, tag="v")
                for sub in range(GROUPS):
                    psv0 = projps.tile([128, 512], F32, tag="projps")
                    psv1 = projps.tile([128, 512], F32, tag="projps")
                    for c in range(4):
                        lx = xt8[:, 2 * c : 2 * c + 2, sub * 128 : (sub + 1) * 128]
                        nc.tensor.matmul(
                            psv0[:], lx, wv[:, 2 * c : 2 * c + 2, 0:512],
                            start=(c == 0), stop=(c == 3), perf_mode=DR,
                        )
                        nc.tensor.matmul(
                            psv1[:], lx, wv[:, 2 * c : 2 * c + 2, 512:1024],
                            start=(c == 0), stop=(c == 3), perf_mode=DR,
                        )
                    nc.scalar.activation(
                        v[:, sub, 0:4, :], psv0.rearrange("p (a b) -> p a b", a=4),
                        mybir.ActivationFunctionType.Copy, scale=1.0 / RS,
                    )
                    nc.scalar.activation(
                        v[:, sub, 4:8, :], psv1.rearrange("p (a b) -> p a b", a=4),
                        mybir.ActivationFunctionType.Copy, scale=1.0 / RS,
                    )

                # ---- attention + output proj + residual + LN per 128-tok group
                mvt = smalls.tile([128, GROUPS, 2], F32, tag="mvt")
                xbs = []
                for g in range(GROUPS):
                    gsl = slice(g * 128, (g + 1) * 128)
                    st = attps.tile([128, H, 128], F32, tag="attps")
                    for h in range(H):
                        nc.tensor.matmul(st[:, h, :], kt[:, h, gsl], qt[:, h, gsl])
                    # add prior/mask (same [128,128] table per head), in place
                    nc.vector.tensor_tensor(
                        st[:], st[:],
                        pm[:, None, :].to_broadcast((128, H, 128)),
                        mybir.AluOpType.add,
                    )
                    pt = ptp.tile([128, H, 128], BF16)
                    nc.scalar.activation(
                        pt[:], st[:], mybir.ActivationFunctionType.Exp
                    )
                    # denominator, broadcast to all partitions: ZB = (1/32)^T P
                    zb0 = projps.tile([128, 512], F32, tag="projps")
                    zb1 = projps.tile([128, 512], F32, tag="projps")
                    nc.tensor.matmul(zb0[:], om[:], pt[:, 0:4, :])
                    nc.tensor.matmul(zb1[:], om[:], pt[:, 4:8, :])
                    rzb = rzbp.tile([128, H, 128], F32)
                    nc.vector.reciprocal_approx_fast(
                        rzb[:, 0:4, :], zb0.rearrange("p (a b) -> p a b", a=4)
                    )
                    nc.vector.reciprocal_approx_fast(
                        rzb[:, 4:8, :], zb1.rearrange("p (a b) -> p a b", a=4)
                    )
                    # OT[d, q] = V^T P per head (lhsT=v), then *32/Z -> fp8
                    ot = otps.tile([128, H, 128], F32, tag="otps")
                    for h in range(H):
                        nc.tensor.matmul(ot[:, h, :], v[:, g, h, :], pt[:, h, :])
                    ot8 = ot8p.tile([128, H, 128], FP8)
                    nc.vector.tensor_tensor(
                        ot8[:], ot[:], rzb[:], mybir.AluOpType.mult
                    )

                    xb = xbp.tile([128, D], F32)
                    nc.sync.dma_start(
                        xb[:], xb_d[tok0 + g * 128 : tok0 + (g + 1) * 128, :]
                    )
                    yp0 = projps.tile([128, 512], F32, tag="projps")
                    yp1 = projps.tile([128, 512], F32, tag="projps")
                    for c in range(4):
                        lo = ot8[:, 2 * c : 2 * c + 2, :]
                        nc.tensor.matmul(
                            yp0[:], lo, wo[:, 2 * c : 2 * c + 2, 0:512],
                            start=(c == 0), stop=(c == 3), perf_mode=DR,
                        )
                        nc.tensor.matmul(
                            yp1[:], lo, wo[:, 2 * c : 2 * c + 2, 512:1024],
                            start=(c == 0), stop=(c == 3), perf_mode=DR,
                        )
                    nc.vector.tensor_tensor(
                        xb[:, 0:512], xb[:, 0:512], yp0[:], mybir.AluOpType.add
                    )
                    nc.vector.tensor_tensor(
                        xb[:, 512:1024], xb[:, 512:1024], yp1[:],
                        mybir.AluOpType.add,
                    )
                    stats = smalls.tile([128, 2, 6], F32, tag="stats")
                    for sg in range(2):
                        nc.vector.bn_stats(
                            stats[:, sg, :], xb[:, sg * 512 : (sg + 1) * 512]
                        )
                    nc.vector.bn_aggr(mvt[:, g, :], stats[:])
                    xbs.append(xb)

                # rstd = exp(-0.5*ln(var+eps')) for the whole tile in one
                # Ln + one Exp: table switches happen per tile, not per group
                sdt = smalls.tile([128, GROUPS], F32, tag="sdt")
                nc.scalar.activation(
                    sdt[:], mvt[:, :, 1],
                    mybir.ActivationFunctionType.Ln, bias=eps[:],
                )
                nc.scalar.activation(
                    sdt[:], sdt[:], mybir.ActivationFunctionType.Exp, scale=-0.5
                )
                for g in range(GROUPS):
                    xb = xbs[g]
                    nc.vector.tensor_scalar(
                        out=xb[:],
                        in0=xb[:],
                        scalar1=mvt[:, g, 0:1],
                        scalar2=sdt[:, g : g + 1],
                        op0=mybir.AluOpType.subtract,
                        op1=mybir.AluOpType.mult,
                    )
                    nc.sync.dma_start(
                        out_d[tok0 + g * 128 : tok0 + (g + 1) * 128, :], xb[:]
                    )

    nc.compile()
    return nc


def _get_nc():
    global _CACHED
    if _CACHED is None:
        _CACHED = _build()
    return _CACHED


def _reference_numpy(modality_encodings, selection_mask, wq, bq, wk, bk, wv, bv,
                     wo, bo, rel_prior, ln_gamma, ln_beta):
    """Slow fallback, exact port of the reference (used only if inputs fall
    outside the fast path's assumptions: non-trivial mask)."""
    x = modality_encodings.astype(np.float32)
    Bn, Kn, Dn = x.shape
    Hd = Dn // H
    q = (x @ wq.T + bq).reshape(Bn, Kn, H, Hd).transpose(0, 2, 1, 3)
    k = (x @ wk.T + bk).reshape(Bn, Kn, H, Hd).transpose(0, 2, 1, 3)
    v = (x @ wv.T + bv).reshape(Bn, Kn, H, Hd).transpose(0, 2, 1, 3)
    scores = np.einsum("bhqd,bhkd->bhqk", q, k) / math.sqrt(Hd)
    scores = scores + rel_prior[None, None]
    mask2d = (selection_mask[:, :, None] * selection_mask[:, None, :]) > 0
    scores = np.where(mask2d[:, None], scores, -np.inf)
    scores = scores - scores.max(axis=-1, keepdims=True)
    e = np.exp(scores)
    attn = e / e.sum(axis=-1, keepdims=True)
    out = np.einsum("bhqk,bhkd->bhqd", attn, v)
    out = out.transpose(0, 2, 1, 3).reshape(Bn, Kn, Dn)
    out = out @ wo.T + bo
    res = x + out
    mu = res.mean(-1, keepdims=True)
    var = ((res - mu) ** 2).mean(-1, keepdims=True)
    return (res - mu) / np.sqrt(var + LN_EPS) * ln_gamma + ln_beta


def _q8(a, scale):
    import ml_dtypes

    return np.clip(a * scale, -240.0, 240.0).astype(ml_dtypes.float8_e4m3)


def _prep_in_maps(modality_encodings, wq, bq, wk, bk, wv, bv, wo, bo, rel_prior):
    import ml_dtypes

    s = 1.0 / math.sqrt(HD)
    wq8 = _q8(np.ascontiguousarray(wq.T), WS)
    wk8 = _q8(np.ascontiguousarray((wk * s).T), KS)
    wv8 = _q8(np.ascontiguousarray(wv.T), WS)
    wo8 = _q8(np.ascontiguousarray(wo.T), WS)
    bks = bk * s
    b_eff = (bo + wo @ bv).astype(np.float32)

    bqk = np.concatenate(
        [bq.reshape(H, HD).T, bks.reshape(H, HD).T], axis=1
    ).astype(np.float32)  # [128, 16]

    pmat = np.full((128, 128), NEG, dtype=np.float32)
    for sm in range(SPG):
        pmat[sm * K : (sm + 1) * K, sm * K : (sm + 1) * K] = rel_prior.T
    omat = np.full((128, 128), 1.0 / XS, dtype=ml_dtypes.bfloat16)

    x_flat = modality_encodings.reshape(B * K, D)
    in_maps = []
    for c in range(N_CORES):
        x_core = x_flat[c * T : (c + 1) * T]
        in_maps.append({
            "XT8": _q8(np.ascontiguousarray(x_core.T), XS),
            "XB": (x_core + b_eff) * RS,
            "WQ8": wq8, "WK8": wk8, "WV8": wv8, "WO8": wo8,
            "BQK": bqk, "PM": pmat, "OM": omat,
        })
    return in_maps


def run_device(inputs, trace=False):
    """Build in_maps from full inputs, run on 8 cores, return (full_out, results)."""
    in_maps = _prep_in_maps(
        inputs["modality_encodings"], inputs["wq"], inputs["bq"], inputs["wk"],
        inputs["bk"], inputs["wv"], inputs["bv"], inputs["wo"], inputs["bo"],
        inputs["rel_prior"],
    )
    nc = _get_nc()
    res = run_bass_kernel_spmd(nc, in_maps, core_ids=list(range(N_CORES)), trace=trace)
    out = np.concatenate(
        [res.results[c]["OUT"].reshape(BC, K, D) for c in range(N_CORES)], axis=0
    )
    return out, res


def kernel(**inputs) -> np.ndarray:
    inputs = {k: np.asarray(v) for k, v in inputs.items()}
    mask = inputs["selection_mask"]
    gamma = inputs["ln_gamma"]
    beta = inputs["ln_beta"]
    if not np.all(mask > 0):
        # general-mask fallback (never hit for the spec'd inputs: fill=ones)
        return _reference_numpy(**{k: inputs[k].astype(np.float32) for k in (
            "modality_encodings", "selection_mask", "wq", "bq", "wk", "bk",
            "wv", "bv", "wo", "bo", "rel_prior", "ln_gamma", "ln_beta")}
        ).astype(np.float32)

    out, _ = run_device(inputs, trace=False)
    # device kernel skips the (identity for spec'd inputs) LN affine params
    if not (np.all(gamma == 1.0) and np.all(beta == 0.0)):
        out = out * gamma + beta
    return out.astype(np.float32)


# revision 11
# speedup vs baseline: 1.9010x; 1.0935x over previous
"""CrossModalityAttention Trainium2 kernel (v2: fp8 DoubleRow projections).

Full inputs -> full output; internally shards batch B=8192 across 8 NeuronCores
(pure data parallel). Per core: 1024 samples x K=8 modalities = 8192 tokens of
D=1024.

Device strategy (per core):
  - All four DxD projections run in fp8-e4m3 with perf_mode=DoubleRow (2
    fp8 MACs/cell/cycle, contraction chunk pairs packed in the AP's dim1).
    Power-of-2 quantization scales: x*32, wq/wv/wo*4096, (wk/sqrt(128))*32768,
    attention-output*32. Descales fold into the ACT bias stage (Q/K/V) or the
    2^17-prescaled residual XB (output proj), whose scale LayerNorm absorbs
    exactly via eps' = eps*2^34 (LN is scale-invariant).
  - Scores per 128-token group (16 samples x K=8) stay bf16:
    ST[(s,k),(s',q)] = Kh^T Qh via matmul(lhsT=kt, rhs=qt); prior/mask table
    adds rel_prior on the block diagonal and -30 off it; exp() -> bf16 probs.
  - Transpose-free tail: OT[d,q] = matmul(lhsT=V[k,d] bf16, rhs=P[k,q] bf16)
    gives the output projection's lhsT directly (no PE transposes). The
    softmax denominator reaches all 128 partitions via one rank-1 PE matmul
    ZB = (1/32 ones)^T @ P, then DVE reciprocal + multiply quantize OT to
    fp8 (the 1/32 folds the fp8 scale for free).
  - LayerNorm: rstd = exp(-0.5*ln(var+eps')) keeps every ACT function in one
    table set.
"""

import math

import numpy as np

import concourse.bacc as bacc
import concourse.bass as bass
import concourse.mybir as mybir
import concourse.tile as tile
from concourse.bass_utils import run_bass_kernel_spmd

N_CORES = 8
B, K, D = 8192, 8, 1024
H, HD = 8, 128
BC = B // N_CORES            # samples per core
T = BC * K                   # tokens per core (8192)
TS = 1024                    # tokens per tile
NT = T // TS                 # tiles per core
GROUPS = TS // 128           # 128-token groups per tile
SPG = 128 // K               # samples per group (16)
LN_EPS = 1e-5
NEG = -30.0                  # large-negative mask for cross-sample scores

XS = 32.0                    # fp8 scale for x and attention output
WS = 4096.0                  # fp8 scale for wq/wv/wo
KS = 32768.0                 # fp8 scale for wk/sqrt(HD)
RS = float(2 ** 17)          # residual prescale (= XS*WS); LN absorbs it

F32 = mybir.dt.float32
BF16 = mybir.dt.bfloat16
FP8 = mybir.dt.float8e4
DR = mybir.MatmulPerfMode.DoubleRow

_CACHED = None  # compiled Bacc module, built once per process


def _build():
    nc = bacc.Bacc("TRN2", target_bir_lowering=False, debug=False, num_devices=1)

    xt8_d = nc.dram_tensor("XT8", [D, T], FP8, kind="ExternalInput").ap()
    xb_d = nc.dram_tensor("XB", [T, D], F32, kind="ExternalInput").ap()
    wq_d = nc.dram_tensor("WQ8", [D, D], FP8, kind="ExternalInput").ap()
    wk_d = nc.dram_tensor("WK8", [D, D], FP8, kind="ExternalInput").ap()
    wv_d = nc.dram_tensor("WV8", [D, D], FP8, kind="ExternalInput").ap()
    wo_d = nc.dram_tensor("WO8", [D, D], FP8, kind="ExternalInput").ap()
    bqk_d = nc.dram_tensor("BQK", [128, 2 * H], F32, kind="ExternalInput").ap()
    pm_d = nc.dram_tensor("PM", [128, 128], F32, kind="ExternalInput").ap()
    om_d = nc.dram_tensor("OM", [128, 128], BF16, kind="ExternalInput").ap()
    out_d = nc.dram_tensor("OUT", [T, D], F32, kind="ExternalOutput").ap()

    xt8_r = xt8_d.rearrange("(c p) t -> p c t", p=128)   # [128, 8, T]

    with tile.TileContext(nc) as tc:
        with (
            tc.tile_pool(name="wpool", bufs=1) as wpool,
            tc.tile_pool(name="consts", bufs=1) as consts,
            tc.tile_pool(name="xt8p", bufs=2) as xt8p,
            tc.tile_pool(name="qkp", bufs=2) as qkp,
            tc.tile_pool(name="vp", bufs=1) as vp,
            tc.tile_pool(name="ptp", bufs=2) as ptp,
            tc.tile_pool(name="rzbp", bufs=2) as rzbp,
            tc.tile_pool(name="ot8p", bufs=2) as ot8p,
            tc.tile_pool(name="xbp", bufs=GROUPS + 1) as xbp,
            tc.tile_pool(name="smalls", bufs=4) as smalls,
            tc.tile_pool(name="projps", bufs=2, space="PSUM") as projps,
            tc.tile_pool(name="attps", bufs=1, space="PSUM") as attps,
            tc.tile_pool(name="otps", bufs=1, space="PSUM") as otps,
        ):
            # ---- constants / weights (resident) ----
            wq = wpool.tile([128, 8, D], FP8, tag="w_q")
            nc.sync.dma_start(wq[:], wq_d.rearrange("(c p) m -> p c m", p=128))
            wk = wpool.tile([128, 8, D], FP8, tag="w_k")
            nc.sync.dma_start(wk[:], wk_d.rearrange("(c p) m -> p c m", p=128))
            wv = wpool.tile([128, 8, D], FP8, tag="w_v")
            nc.sync.dma_start(wv[:], wv_d.rearrange("(c p) m -> p c m", p=128))
            wo = wpool.tile([128, 8, D], FP8, tag="w_o")
            nc.sync.dma_start(wo[:], wo_d.rearrange("(c p) m -> p c m", p=128))
            bqk = consts.tile([128, 2 * H], F32)
            nc.sync.dma_start(bqk[:], bqk_d)
            pm = consts.tile([128, 128], F32)
            nc.sync.dma_start(pm[:], pm_d)
            om = consts.tile([128, 128], BF16)
            nc.sync.dma_start(om[:], om_d)
            eps = consts.tile([128, 1], F32)
            nc.vector.memset(eps[:], LN_EPS * RS * RS)

            for t in range(NT):
                tok0 = t * TS
                xt8 = xt8p.tile([128, 8, TS], FP8)
                nc.sync.dma_start(xt8[:], xt8_r[:, :, tok0 : tok0 + TS])

                # ---- Q^T, K^T projections (fp8 DoubleRow): [d_head x tok]
                qt = qkp.tile([128, H, TS], BF16, tag="qt")
                kt = qkp.tile([128, H, TS], BF16, tag="kt")
                for wt, dst, bias_col0, dsc in (
                    (wq, qt, 0, 1.0 / RS),
                    (wk, kt, H, 1.0 / (KS * XS)),
                ):
                    for h in range(H):
                        psab = projps.tile([128, 1024], F32, tag="projps")
                        for c in range(4):
                            lw = wt[:, 2 * c : 2 * c + 2, h * HD : (h + 1) * HD]
                            nc.tensor.matmul(
                                psab[:, 0:512], lw, xt8[:, 2 * c : 2 * c + 2, 0:512],
                                start=(c == 0), stop=(c == 3), perf_mode=DR,
                            )
                            nc.tensor.matmul(
                                psab[:, 512:1024], lw,
                                xt8[:, 2 * c : 2 * c + 2, 512:1024],
                                start=(c == 0), stop=(c == 3), perf_mode=DR,
                            )
                        nc.scalar.activation(
                            dst[:, h, :], psab[:],
                            mybir.ActivationFunctionType.Identity,
                            bias=bqk[:, bias_col0 + h : bias_col0 + h + 1],
                            scale=dsc,
                        )

                # ---- V projection (fp8 DoubleRow), token-major bf16
                v = vp.tile([128, GROUPS, H, HD], BF16, tag="v")
                for sub in range(GROUPS):
                    psv = projps.tile([128, 1024], F32, tag="projps")
                    for c in range(4):
                        lx = xt8[:, 2 * c : 2 * c + 2, sub * 128 : (sub + 1) * 128]
                        nc.tensor.matmul(
                            psv[:, 0:512], lx, wv[:, 2 * c : 2 * c + 2, 0:512],
                            start=(c == 0), stop=(c == 3), perf_mode=DR,
                        )
                        nc.tensor.matmul(
                            psv[:, 512:1024], lx, wv[:, 2 * c : 2 * c + 2, 512:1024],
                            start=(c == 0), stop=(c == 3), perf_mode=DR,
                        )
                    nc.scalar.activation(
                        v[:, sub, :, :], psv.rearrange("p (a b) -> p a b", a=8),
                        mybir.ActivationFunctionType.Copy, scale=1.0 / RS,
                    )

                # ---- attention + output proj + residual + LN per 128-tok group
                mvt = smalls.tile([128, GROUPS, 2], F32, tag="mvt")
                xbs = []
                for g in range(GROUPS):
                    gsl = slice(g * 128, (g + 1) * 128)
                    st = attps.tile([128, H, 128], F32, tag="attps")
                    for h in range(H):
                        nc.tensor.matmul(st[:, h, :], kt[:, h, gsl], qt[:, h, gsl])
                    # add prior/mask (same [128,128] table per head), in place
                    nc.vector.tensor_tensor(
                        st[:], st[:],
                        pm[:, None, :].to_broadcast((128, H, 128)),
                        mybir.AluOpType.add,
                    )
                    pt = ptp.tile([128, H, 128], BF16)
                    nc.scalar.activation(
                        pt[:], st[:], mybir.ActivationFunctionType.Exp
                    )
                    # denominator, broadcast to all partitions: ZB = (1/32)^T P
                    zb = projps.tile([128, 1024], F32, tag="projps")
                    nc.tensor.matmul(zb[:, 0:512], om[:], pt[:, 0:4, :])
                    nc.tensor.matmul(zb[:, 512:1024], om[:], pt[:, 4:8, :])
                    rzb = rzbp.tile([128, H, 128], F32)
                    nc.vector.reciprocal_approx_fast(
                        rzb[:], zb.rearrange("p (a b) -> p a b", a=8)
                    )
                    # OT[d, q] = V^T P per head (lhsT=v), then *32/Z -> fp8
                    ot = otps.tile([128, H, 128], F32, tag="otps")
                    for h in range(H):
                        nc.tensor.matmul(ot[:, h, :], v[:, g, h, :], pt[:, h, :])
                    ot8 = ot8p.tile([128, H, 128], FP8)
                    nc.vector.tensor_tensor(
                        ot8[:], ot[:], rzb[:], mybir.AluOpType.mult
                    )

                    xb = xbp.tile([128, D], F32)
                    nc.sync.dma_start(
                        xb[:], xb_d[tok0 + g * 128 : tok0 + (g + 1) * 128, :]
                    )
                    yp = projps.tile([128, 1024], F32, tag="projps")
                    for c in range(4):
                        lo = ot8[:, 2 * c : 2 * c + 2, :]
                        nc.tensor.matmul(
                            yp[:, 0:512], lo, wo[:, 2 * c : 2 * c + 2, 0:512],
                            start=(c == 0), stop=(c == 3), perf_mode=DR,
                        )
                        nc.tensor.matmul(
                            yp[:, 512:1024], lo, wo[:, 2 * c : 2 * c + 2, 512:1024],
                            start=(c == 0), stop=(c == 3), perf_mode=DR,
                        )
                    nc.vector.tensor_tensor(
                        xb[:], xb[:], yp[:], mybir.AluOpType.add
                    )
                    stats = smalls.tile([128, 2, 6], F32, tag="stats")
                    for sg in range(2):
                        nc.vector.bn_stats(
                            stats[:, sg, :], xb[:, sg * 512 : (sg + 1) * 512]
                        )
                    nc.vector.bn_aggr(mvt[:, g, :], stats[:])
                    xbs.append(xb)

                # rstd = exp(-0.5*ln(var+eps')) for the whole tile in one
                # Ln + one Exp: table switches happen per tile, not per group
                sdt = smalls.tile([128, GROUPS], F32, tag="sdt")
                nc.scalar.activation(
                    sdt[:], mvt[:, :, 1],
                    mybir.ActivationFunctionType.Ln, bias=eps[:],
                )
                nc.scalar.activation(
                    sdt[:], sdt[:], mybir.ActivationFunctionType.Exp, scale=-0.5
                )
                for g in range(GROUPS):
                    xb = xbs[g]
                    nc.vector.tensor_scalar(
                        out=xb[:],
                        in0=xb[:],
                        scalar1=mvt[:, g, 0:1],
                        scalar2=sdt[:, g : g + 1],
                        op0=mybir.AluOpType.subtract,
                        op1=mybir.AluOpType.mult,
                    )
                    nc.sync.dma_start(
                        out_d[tok0 + g * 128 : tok0 + (g + 1) * 128, :], xb[:]
                    )

    nc.compile()
    return nc


def _get_nc():
    global _CACHED
    if _CACHED is None:
        _CACHED = _build()
    return _CACHED


def _reference_numpy(modality_encodings, selection_mask, wq, bq, wk, bk, wv, bv,
                     wo, bo, rel_prior, ln_gamma, ln_beta):
    """Slow fallback, exact port of the reference (used only if inputs fall
    outside the fast path's assumptions: non-trivial mask)."""
    x = modality_encodings.astype(np.float32)
    Bn, Kn, Dn = x.shape
    Hd = Dn // H
    q = (x @ wq.T + bq).reshape(Bn, Kn, H, Hd).transpose(0, 2, 1, 3)
    k = (x @ wk.T + bk).reshape(Bn, Kn, H, Hd).transpose(0, 2, 1, 3)
    v = (x @ wv.T + bv).reshape(Bn, Kn, H, Hd).transpose(0, 2, 1, 3)
    scores = np.einsum("bhqd,bhkd->bhqk", q, k) / math.sqrt(Hd)
    scores = scores + rel_prior[None, None]
    mask2d = (selection_mask[:, :, None] * selection_mask[:, None, :]) > 0
    scores = np.where(mask2d[:, None], scores, -np.inf)
    scores = scores - scores.max(axis=-1, keepdims=True)
    e = np.exp(scores)
    attn = e / e.sum(axis=-1, keepdims=True)
    out = np.einsum("bhqk,bhkd->bhqd", attn, v)
    out = out.transpose(0, 2, 1, 3).reshape(Bn, Kn, Dn)
    out = out @ wo.T + bo
    res = x + out
    mu = res.mean(-1, keepdims=True)
    var = ((res - mu) ** 2).mean(-1, keepdims=True)
    return (res - mu) / np.sqrt(var + LN_EPS) * ln_gamma + ln_beta


def _q8(a, scale):
    import ml_dtypes

    return np.clip(a * scale, -240.0, 240.0).astype(ml_dtypes.float8_e4m3)


def _prep_in_maps(modality_encodings, wq, bq, wk, bk, wv, bv, wo, bo, rel_prior):
    import ml_dtypes

    s = 1.0 / math.sqrt(HD)
    wq8 = _q8(np.ascontiguousarray(wq.T), WS)
    wk8 = _q8(np.ascontiguousarray((wk * s).T), KS)
    wv8 = _q8(np.ascontiguousarray(wv.T), WS)
    wo8 = _q8(np.ascontiguousarray(wo.T), WS)
    bks = bk * s
    b_eff = (bo + wo @ bv).astype(np.float32)

    bqk = np.concatenate(
        [bq.reshape(H, HD).T, bks.reshape(H, HD).T], axis=1
    ).astype(np.float32)  # [128, 16]

    pmat = np.full((128, 128), NEG, dtype=np.float32)
    for sm in range(SPG):
        pmat[sm * K : (sm + 1) * K, sm * K : (sm + 1) * K] = rel_prior.T
    omat = np.full((128, 128), 1.0 / XS, dtype=ml_dtypes.bfloat16)

    x_flat = modality_encodings.reshape(B * K, D)
    in_maps = []
    for c in range(N_CORES):
        x_core = x_flat[c * T : (c + 1) * T]
        in_maps.append({
            "XT8": _q8(np.ascontiguousarray(x_core.T), XS),
            "XB": (x_core + b_eff) * RS,
            "WQ8": wq8, "WK8": wk8, "WV8": wv8, "WO8": wo8,
            "BQK": bqk, "PM": pmat, "OM": omat,
        })
    return in_maps


def run_device(inputs, trace=False):
    """Build in_maps from full inputs, run on 8 cores, return (full_out, results)."""
    in_maps = _prep_in_maps(
        inputs["modality_encodings"], inputs["wq"], inputs["bq"], inputs["wk"],
        inputs["bk"], inputs["wv"], inputs["bv"], inputs["wo"], inputs["bo"],
        inputs["rel_prior"],
    )
    nc = _get_nc()
    res = run_bass_kernel_spmd(nc, in_maps, core_ids=list(range(N_CORES)), trace=trace)
    out = np.concatenate(
        [res.results[c]["OUT"].reshape(BC, K, D) for c in range(N_CORES)], axis=0
    )
    return out, res


def kernel(**inputs) -> np.ndarray:
    inputs = {k: np.asarray(v) for k, v in inputs.items()}
    mask = inputs["selection_mask"]
    gamma = inputs["ln_gamma"]
    beta = inputs["ln_beta"]
    if not np.all(mask > 0):
        # general-mask fallback (never hit for the spec'd inputs: fill=ones)
        return _reference_numpy(**{k: inputs[k].astype(np.float32) for k in (
            "modality_encodings", "selection_mask", "wq", "bq", "wk", "bk",
            "wv", "bv", "wo", "bo", "rel_prior", "ln_gamma", "ln_beta")}
        ).astype(np.float32)

    out, _ = run_device(inputs, trace=False)
    # device kernel skips the (identity for spec'd inputs) LN affine params
    if not (np.all(gamma == 1.0) and np.all(beta == 0.0)):
        out = out * gamma + beta
    return out.astype(np.float32)
